# revision 12
# baseline (speedup 1.0000x reference)
"""v4: bf16 GEMM + fp8-DoubleRow tail tiles, resident weights, bf16 out.

y = ((x * tmp_L) @ W^T) * tmp_R + bias, data-parallel over B=8 cores.
Per core: [T=1024, NX=1024] @ [NX, NF=4096].
- FTB nf-tiles (ft) in bf16 (1 col/cycle), NF8 in fp8e4m3 DoubleRow (2x).
- Host folds tmp_L into x; tmp_R/bias applied by scalar ACT on PSUM evict.
- Output bf16, upcast on host (rel-err cost ~1e-4).
DMA notes (measured): per-queue DMA is packet-bound (~128 packets per
[128,*] transfer, ~35 packets/us solo, less under contention), so x and
weights use few DMAs with long per-partition runs; tr/bt combined into
one tensor to avoid a slow 128B-packet transfer blocking the x queue.
exec_time ends at last output DMA completion, so the final ft's output
is split by partition halves across two queues.
"""

import numpy as np
import ml_dtypes

import concourse.bacc as bacc
import concourse.mybir as mybir
import concourse.tile as tile
from concourse.bass_utils import run_bass_kernel_spmd

B, T, NX, NF, KC = 8, 1024, 1024, 4096, 50
N_CORES = 8
P = 128
KT = NX // P          # 8 contraction tiles
FT = NF // P          # 32 output tiles
NF8 = 6               # trailing fts computed in fp8 DoubleRow
FTB = FT - NF8        # leading fts in bf16
TCH = 512
NTC = T // TCH
# bf16 weight DMA groups: (f0, f1, queue). Sizes ramp so ft0 lands
# first; spread across queues to bound contention.
WGRPS = [(0, 2, "gpsimd"), (2, 6, "gpsimd"), (6, 10, "scalar"),
         (10, 18, "sync"), (18, FTB, "gpsimd")]

F32 = mybir.dt.float32
BF16 = mybir.dt.bfloat16
FP8 = mybir.dt.float8e4
DR = mybir.MatmulPerfMode.DoubleRow

TRACE = False
LAST_RESULT = None

_cached = None


def _build():
    nc = bacc.Bacc("TRN2", target_bir_lowering=False, debug=False,
                   num_devices=N_CORES)

    xh = nc.dram_tensor("xh", [P, KT, T], BF16, kind="ExternalInput").ap()
    xh8 = nc.dram_tensor("xh8", [P, KT, T], FP8, kind="ExternalInput").ap()
    wt = nc.dram_tensor("wt", [P, FTB, KT, P], BF16,
                        kind="ExternalInput").ap()
    wt8 = nc.dram_tensor("wt8", [P, NF8, KT, P], FP8,
                         kind="ExternalInput").ap()
    trbt = nc.dram_tensor("trbt", [P, 2, FT], F32, kind="ExternalInput").ap()
    ot = nc.dram_tensor("ot", [FT, P, T], BF16, kind="ExternalOutput").ap()

    with tile.TileContext(nc) as tc:
        with (
            tc.tile_pool(name="const", bufs=1) as cpool,
            tc.tile_pool(name="opool", bufs=6) as opool,
            tc.tile_pool(name="psacc", bufs=4, space="PSUM") as pspool,
        ):
            # memset first so PE warm-up can start asap
            warm = cpool.tile([P, TCH], BF16)
            nc.gpsimd.memset(warm, 0.0)

            # trbt first on scalar (tiny; must beat the first ACT)
            trbt_sb = cpool.tile([P, 2, FT], F32)
            nc.scalar.dma_start(out=trbt_sb, in_=trbt)
            tr_sb = trbt_sb[:, 0, :]
            bias_sb = trbt_sb[:, 1, :]

            # x: tiles per k-pair, spread over scalar+sync
            xs_j = [cpool.tile([P, 2, T], BF16, name=f"xsj{j}")
                    for j in range(KT // 2)]
            nc.scalar.dma_start(out=xs_j[0], in_=xh[:, 0:2, :])
            nc.sync.dma_start(out=xs_j[1], in_=xh[:, 2:4, :])
            nc.scalar.dma_start(out=xs_j[2], in_=xh[:, 4:6, :])
            nc.sync.dma_start(out=xs_j[3], in_=xh[:, 6:8, :])

            # weights fully resident
            qmap = {"gpsimd": nc.gpsimd, "scalar": nc.scalar,
                    "sync": nc.sync}
            w_g = {}
            for (f0, f1, q) in WGRPS:
                t_ = cpool.tile([P, f1 - f0, KT, P], BF16, name=f"wg{f0}")
                qmap[q].dma_start(out=t_, in_=wt[:, f0:f1])
                w_g[(f0, f1)] = t_
            w8_sb = cpool.tile([P, NF8, KT, P], FP8, name="w8")
            nc.sync.dma_start(out=w8_sb, in_=wt8)
            # fp8 x (needed late): single tile, one DMA
            xs8_sb = cpool.tile([P, KT, T], FP8, name="xs8")
            nc.gpsimd.dma_start(out=xs8_sb, in_=xh8)

            def wtile(ft):
                for (f0, f1), t_ in w_g.items():
                    if f0 <= ft < f1:
                        return t_, ft - f0
                raise AssertionError(ft)

            # PE warm-up: ramp DVFS while input DMAs land
            for _ in range(6):
                wps = pspool.tile([P, TCH], F32, tag="acc", bufs=4,
                                  name="warm_ps")
                nc.tensor.matmul(wps, lhsT=warm[:, :P], rhs=warm,
                                 start=True, stop=True)

            for ft in range(FT):
                out_sb = opool.tile([P, T], BF16, tag="out")
                is8 = ft >= FTB
                last = ft == FT - 1
                ntc, tch = (4, T // 4) if last else (NTC, TCH)
                for tci in range(ntc):
                    sl = slice(tci * tch, (tci + 1) * tch)
                    ps = pspool.tile([P, tch], F32,
                                     tag="accq" if last else "acc", bufs=4)
                    if is8:
                        f8 = ft - FTB
                        for j in range(KT // 2):
                            nc.tensor.matmul(
                                ps,
                                lhsT=w8_sb[:, f8, 2 * j:2 * j + 2, :],
                                rhs=xs8_sb[:, 2 * j:2 * j + 2, sl],
                                start=(j == 0), stop=(j == KT // 2 - 1),
                                perf_mode=DR,
                            )
                    else:
                        wg, fl = wtile(ft)
                        for k in range(KT):
                            nc.tensor.matmul(
                                ps,
                                lhsT=wg[:, fl, k, :],
                                rhs=xs_j[k // 2][:, k % 2, sl],
                                start=(k == 0), stop=(k == KT - 1),
                            )
                    nc.scalar.activation(
                        out_sb[:, sl], ps,
                        mybir.ActivationFunctionType.Identity,
                        bias=bias_sb[:, ft:ft + 1],
                        scale=tr_sb[:, ft:ft + 1],
                    )
                    if last:
                        # split by partition halves over both queues: the
                        # final transfer bounds exec end time
                        nc.sync.dma_start(out=ot[ft, :P // 2, sl],
                                          in_=out_sb[:P // 2, sl])
                        nc.gpsimd.dma_start(out=ot[ft, P // 2:, sl],
                                            in_=out_sb[P // 2:, sl])
                if not last:
                    if is8:
                        # fp8 fts complete 2x faster than one queue can
                        # drain 256KB: split each across both queues
                        nc.sync.dma_start(out=ot[ft, :P // 2],
                                          in_=out_sb[:P // 2])
                        nc.gpsimd.dma_start(out=ot[ft, P // 2:],
                                            in_=out_sb[P // 2:])
                    else:
                        q = nc.sync if ft % 2 == 0 else nc.gpsimd
                        q.dma_start(out=ot[ft], in_=out_sb)

    nc.compile()
    return nc


def kernel(x, cluster, weight, bias, style_L, style_R):
    global _cached, LAST_RESULT
    x = np.ascontiguousarray(np.asarray(x, dtype=np.float32))
    cluster = np.ascontiguousarray(np.asarray(cluster, dtype=np.float32))
    weight = np.ascontiguousarray(np.asarray(weight, dtype=np.float32))
    bias = np.ascontiguousarray(np.asarray(bias, dtype=np.float32))
    style_L = np.ascontiguousarray(np.asarray(style_L, dtype=np.float32))
    style_R = np.ascontiguousarray(np.asarray(style_R, dtype=np.float32))

    if _cached is None:
        _cached = _build()
    nc = _cached

    tmp_L = cluster @ style_L
    tmp_R = cluster @ style_R
    xs = x * tmp_L[:, None, :]
    # [B, T, KT, P] -> [B, P, KT, T]
    xs4 = xs.reshape(B, T, KT, P).transpose(0, 3, 2, 1)
    xh_all = np.ascontiguousarray(xs4.astype(ml_dtypes.bfloat16))
    xh8_all = np.ascontiguousarray(xs4.astype(ml_dtypes.float8_e4m3))
    # weight [NF, NX] -> [FT, Pf, KT, Px] -> [Px, FT, KT, Pf]
    w4 = weight.reshape(FT, P, KT, P).transpose(3, 0, 2, 1)
    wt_h = np.ascontiguousarray(w4[:, :FTB].astype(ml_dtypes.bfloat16))
    wt8_h = np.ascontiguousarray(w4[:, FTB:].astype(ml_dtypes.float8_e4m3))
    # [B, P, 2, FT]: dim2 = (tmp_R, bias)
    trc = tmp_R.reshape(B, FT, P).transpose(0, 2, 1)
    btc = np.broadcast_to(bias.reshape(FT, P).T, (B, P, FT))
    trbt_h = np.ascontiguousarray(
        np.stack([trc, btc], axis=2).astype(np.float32))

    in_maps = [
        {"xh": xh_all[c], "xh8": xh8_all[c], "wt": wt_h, "wt8": wt8_h,
         "trbt": trbt_h[c]}
        for c in range(N_CORES)
    ]

    res = run_bass_kernel_spmd(nc, in_maps, core_ids=list(range(N_CORES)),
                               trace=TRACE)
    LAST_RESULT = res

    out = np.empty((B, T, NF), dtype=np.float32)
    for c in range(N_CORES):
        otc = np.asarray(res.results[c]["ot"]).astype(np.float32)
        out[c] = otc.transpose(2, 0, 1).reshape(T, NF)
    return out


# revision 17
# speedup vs baseline: 1.1229x; 1.1229x over previous
"""v4: bf16 GEMM + fp8-DoubleRow tail tiles, resident weights, bf16 out.

y = ((x * tmp_L) @ W^T) * tmp_R + bias, data-parallel over B=8 cores.
Per core: [T=1024, NX=1024] @ [NX, NF=4096].
- FTB nf-tiles (ft) in bf16 (1 col/cycle), NF8 in fp8e4m3 DoubleRow (2x).
- Host folds tmp_L into x; tmp_R/bias applied by scalar ACT on PSUM evict.
- Output bf16, upcast on host (rel-err cost ~1e-4).
DMA notes (measured): per-queue DMA is packet-bound (~128 packets per
[128,*] transfer, ~35 packets/us solo, less under contention), so x and
weights use few DMAs with long per-partition runs; tr/bt combined into
one tensor to avoid a slow 128B-packet transfer blocking the x queue.
exec_time ends at last output DMA completion, so the final ft's output
is split by partition halves across two queues.
"""

import numpy as np
import ml_dtypes

import concourse.bacc as bacc
import concourse.mybir as mybir
import concourse.tile as tile
from concourse.bass_utils import run_bass_kernel_spmd

B, T, NX, NF, KC = 8, 1024, 1024, 4096, 50
N_CORES = 8
P = 128
KT = NX // P          # 8 contraction tiles
FT = NF // P          # 32 output tiles
NF8 = 6               # trailing fts computed in fp8 DoubleRow
FTB = FT - NF8        # leading fts in bf16
TCH = 512
NTC = T // TCH
# bf16 weight DMA groups: (f0, f1, queue). Sizes ramp so ft0 lands
# first. NOTE: the scalar queue must carry no bulk DMA — a queue's next
# DMA trigger blocks until the previous transfer completes, which would
# stall the ACTs queued behind it.
WGRPS = [(0, 2, "gpsimd"), (2, 6, "gpsimd"), (6, 10, "gpsimd"),
         (10, 18, "sync"), (18, FTB, "gpsimd")]

F32 = mybir.dt.float32
BF16 = mybir.dt.bfloat16
FP8 = mybir.dt.float8e4
DR = mybir.MatmulPerfMode.DoubleRow

TRACE = False
LAST_RESULT = None

_cached = None


def _build():
    nc = bacc.Bacc("TRN2", target_bir_lowering=False, debug=False,
                   num_devices=N_CORES)

    xh = nc.dram_tensor("xh", [P, NTC, KT, TCH], BF16,
                        kind="ExternalInput").ap()
    xh8 = nc.dram_tensor("xh8", [P, KT, T], FP8, kind="ExternalInput").ap()
    wt = nc.dram_tensor("wt", [P, FTB, KT, P], BF16,
                        kind="ExternalInput").ap()
    wt8 = nc.dram_tensor("wt8", [P, NF8, KT, P], FP8,
                         kind="ExternalInput").ap()
    trbt = nc.dram_tensor("trbt", [P, 2, FT], F32, kind="ExternalInput").ap()
    ot = nc.dram_tensor("ot", [FT, P, T], BF16, kind="ExternalOutput").ap()

    with tile.TileContext(nc) as tc:
        with (
            tc.tile_pool(name="const", bufs=1) as cpool,
            tc.tile_pool(name="opool", bufs=6) as opool,
            tc.tile_pool(name="psacc", bufs=4, space="PSUM") as pspool,
        ):
            # memset first so PE warm-up can start asap
            warm = cpool.tile([P, TCH], BF16)
            nc.gpsimd.memset(warm, 0.0)

            # trbt first on scalar (tiny; must beat the first ACT)
            trbt_sb = cpool.tile([P, 2, FT], F32)
            nc.scalar.dma_start(out=trbt_sb, in_=trbt)
            tr_sb = trbt_sb[:, 0, :]
            bias_sb = trbt_sb[:, 1, :]

            # x: one tile per T-chunk (all k together), both on sync so
            # the first chunk's chain is fully fed as soon as it lands
            xs_t = [cpool.tile([P, KT, TCH], BF16, name=f"xst{t}")
                    for t in range(NTC)]
            nc.sync.dma_start(out=xs_t[0], in_=xh[:, 0])
            nc.sync.dma_start(out=xs_t[1], in_=xh[:, 1])

            # weights fully resident
            qmap = {"gpsimd": nc.gpsimd, "scalar": nc.scalar,
                    "sync": nc.sync}
            w_g = {}
            for (f0, f1, q) in WGRPS:
                t_ = cpool.tile([P, f1 - f0, KT, P], BF16, name=f"wg{f0}")
                qmap[q].dma_start(out=t_, in_=wt[:, f0:f1])
                w_g[(f0, f1)] = t_
            w8_sb = cpool.tile([P, NF8, KT, P], FP8, name="w8")
            nc.sync.dma_start(out=w8_sb, in_=wt8)
            # fp8 x (needed late): single tile, one DMA
            xs8_sb = cpool.tile([P, KT, T], FP8, name="xs8")
            nc.gpsimd.dma_start(out=xs8_sb, in_=xh8)

            def wtile(ft):
                for (f0, f1), t_ in w_g.items():
                    if f0 <= ft < f1:
                        return t_, ft - f0
                raise AssertionError(ft)

            # PE warm-up: ramp DVFS while input DMAs land
            for _ in range(6):
                wps = pspool.tile([P, TCH], F32, tag="acc", bufs=4,
                                  name="warm_ps")
                nc.tensor.matmul(wps, lhsT=warm[:, :P], rhs=warm,
                                 start=True, stop=True)

            for ft in range(FT):
                out_sb = opool.tile([P, T], BF16, tag="out")
                is8 = ft >= FTB
                last = ft == FT - 1
                ntc, tch = (4, T // 4) if last else (NTC, TCH)
                for tci in range(ntc):
                    sl = slice(tci * tch, (tci + 1) * tch)
                    ps = pspool.tile([P, tch], F32,
                                     tag="accq" if last else "acc", bufs=4)
                    if is8:
                        f8 = ft - FTB
                        for j in range(KT // 2):
                            nc.tensor.matmul(
                                ps,
                                lhsT=w8_sb[:, f8, 2 * j:2 * j + 2, :],
                                rhs=xs8_sb[:, 2 * j:2 * j + 2, sl],
                                start=(j == 0), stop=(j == KT // 2 - 1),
                                perf_mode=DR,
                            )
                    else:
                        wg, fl = wtile(ft)
                        for k in range(KT):
                            nc.tensor.matmul(
                                ps,
                                lhsT=wg[:, fl, k, :],
                                rhs=xs_t[tci][:, k, :],
                                start=(k == 0), stop=(k == KT - 1),
                            )
                    nc.scalar.activation(
                        out_sb[:, sl], ps,
                        mybir.ActivationFunctionType.Identity,
                        bias=bias_sb[:, ft:ft + 1],
                        scale=tr_sb[:, ft:ft + 1],
                    )
                    if last:
                        # split by partition halves over both queues: the
                        # final transfer bounds exec end time
                        nc.sync.dma_start(out=ot[ft, :P // 2, sl],
                                          in_=out_sb[:P // 2, sl])
                        nc.gpsimd.dma_start(out=ot[ft, P // 2:, sl],
                                            in_=out_sb[P // 2:, sl])
                if not last:
                    if is8:
                        # fp8 fts complete 2x faster than one queue can
                        # drain 256KB: split each across both queues
                        nc.sync.dma_start(out=ot[ft, :P // 2],
                                          in_=out_sb[:P // 2])
                        nc.gpsimd.dma_start(out=ot[ft, P // 2:],
                                            in_=out_sb[P // 2:])
                    else:
                        q = nc.sync if ft % 2 == 0 else nc.gpsimd
                        q.dma_start(out=ot[ft], in_=out_sb)

    nc.compile()
    return nc


def kernel(x, cluster, weight, bias, style_L, style_R):
    global _cached, LAST_RESULT
    x = np.ascontiguousarray(np.asarray(x, dtype=np.float32))
    cluster = np.ascontiguousarray(np.asarray(cluster, dtype=np.float32))
    weight = np.ascontiguousarray(np.asarray(weight, dtype=np.float32))
    bias = np.ascontiguousarray(np.asarray(bias, dtype=np.float32))
    style_L = np.ascontiguousarray(np.asarray(style_L, dtype=np.float32))
    style_R = np.ascontiguousarray(np.asarray(style_R, dtype=np.float32))

    if _cached is None:
        _cached = _build()
    nc = _cached

    tmp_L = cluster @ style_L
    tmp_R = cluster @ style_R
    xs = x * tmp_L[:, None, :]
    # [B, T, KT, P] -> [B, P, KT, T]
    xs4 = xs.reshape(B, T, KT, P).transpose(0, 3, 2, 1)
    # bf16 x in tci-major layout: [B, P, NTC, KT, TCH]
    xs5 = xs4.reshape(B, P, KT, NTC, TCH).transpose(0, 1, 3, 2, 4)
    xh_all = np.ascontiguousarray(xs5.astype(ml_dtypes.bfloat16))
    xh8_all = np.ascontiguousarray(xs4.astype(ml_dtypes.float8_e4m3))
    # weight [NF, NX] -> [FT, Pf, KT, Px] -> [Px, FT, KT, Pf]
    w4 = weight.reshape(FT, P, KT, P).transpose(3, 0, 2, 1)
    wt_h = np.ascontiguousarray(w4[:, :FTB].astype(ml_dtypes.bfloat16))
    wt8_h = np.ascontiguousarray(w4[:, FTB:].astype(ml_dtypes.float8_e4m3))
    # [B, P, 2, FT]: dim2 = (tmp_R, bias)
    trc = tmp_R.reshape(B, FT, P).transpose(0, 2, 1)
    btc = np.broadcast_to(bias.reshape(FT, P).T, (B, P, FT))
    trbt_h = np.ascontiguousarray(
        np.stack([trc, btc], axis=2).astype(np.float32))

    in_maps = [
        {"xh": xh_all[c], "xh8": xh8_all[c], "wt": wt_h, "wt8": wt8_h,
         "trbt": trbt_h[c]}
        for c in range(N_CORES)
    ]

    res = run_bass_kernel_spmd(nc, in_maps, core_ids=list(range(N_CORES)),
                               trace=TRACE)
    LAST_RESULT = res

    out = np.empty((B, T, NF), dtype=np.float32)
    for c in range(N_CORES):
        otc = np.asarray(res.results[c]["ot"]).astype(np.float32)
        out[c] = otc.transpose(2, 0, 1).reshape(T, NF)
    return out
